# revision 26
# baseline (speedup 1.0000x reference)
"""Trainium2 Bass kernel for nn_DynamicSemanticHead (MoE-routed dynamic 3^3 conv).

Sharding: 8 cores = 2 (batch) x 4 (z-slabs of 24 output slices).
Per-core formulation: conv as banded matmuls.
  K = 128 partitions = 8 staged input z-slices x 16 cin
  M = 48 psum partitions = 6 output z-slices x 8 cout
  free = contiguous (y,x) plane (96*96) with zero guard zones; y/x taps are
  free-dim shifts. z-taps live in the banded lhsT. x-shifts wrap across row
  boundaries; the wrapped contributions are subtracted via small edge-column
  correction matmuls (columns x=0 and x=95).
Router/modulator MLPs, softmax and expert weight mixing run on-device.
"""
import sys

for _p in ("/opt/trn_rl_repo",):
    if _p not in sys.path:
        sys.path.append(_p)

import numpy as np

import concourse.bass as bass
import concourse.tile as tile
from concourse import bacc, mybir
from concourse.bass_utils import run_bass_kernel_spmd

F32 = mybir.dt.float32
F32R = mybir.dt.float32r

B, CIN, COUT, D, TDIM, E, HID = 2, 16, 8, 96, 256, 8, 64
NCORES = 8
SLAB = 24          # output z slices per core
ZB = 4             # z blocks per core
QZ = 6             # output z per block
RZ = 8             # staged input slices per block
NSLICE = 26        # x_slab z extent (24 + 2 halo)
G = 100            # guard columns each side of the (y,x) plane
SL = D * D         # 9216, one slice plane
XW = G + SL + G    # staged width per slice
M = QZ * COUT      # 48
RPT = 5            # output y rows per psum tile
NT = 20            # 19 tiles of 5 rows + 1 tile of 1 row
TILESET = 5
EG = 4             # guard for edge-column buffers
EW = EG + D + EG

_CACHE = {}
DEBUG = False


def _pack_ap(t, offset, dims):
    """dims = [(step, count), ...] in flat element space of t's tensor.
    Only the first dim may address partitions (SBUF)."""
    return bass.AP(tensor=t.tensor, offset=t.offset + offset,
                   ap=[[s, c] for (s, c) in dims])


def _build_program():
    nc = bacc.Bacc("TRN2", target_bir_lowering=False, debug=False,
                   num_devices=NCORES)

    xs = nc.declare_dram_parameter("x_slab", [CIN, NSLICE, D, D], F32R, isOutput=False)
    text2 = nc.declare_dram_parameter("text2", [128, 2], F32, isOutput=False)
    rw1 = nc.declare_dram_parameter("r_w1", [TDIM, HID], F32, isOutput=False)
    rb1 = nc.declare_dram_parameter("r_b1", [HID], F32, isOutput=False)
    rw2 = nc.declare_dram_parameter("r_w2", [HID, E], F32, isOutput=False)
    rb2 = nc.declare_dram_parameter("r_b2", [E], F32, isOutput=False)
    mw1 = nc.declare_dram_parameter("m_w1", [TDIM, HID], F32, isOutput=False)
    mb1 = nc.declare_dram_parameter("m_b1", [HID], F32, isOutput=False)
    mw2 = nc.declare_dram_parameter("m_w2", [HID, COUT], F32, isOutput=False)
    mb2 = nc.declare_dram_parameter("m_b2", [COUT], F32, isOutput=False)
    basis = nc.declare_dram_parameter("basis", [E, COUT * CIN * 27], F32, isOutput=False)
    bbas = nc.declare_dram_parameter("bias_basis", [E, COUT], F32, isOutput=False)

    out_d = nc.declare_dram_parameter("out", [COUT, SLAB, D, D], F32, isOutput=True)
    logits_d = nc.declare_dram_parameter("logits", [1, E], F32, isOutput=True)
    rwts_d = nc.declare_dram_parameter("rweights", [1, E], F32, isOutput=True)

    from contextlib import ExitStack
    with tile.TileContext(nc) as tc, ExitStack() as st:
        singles = st.enter_context(tc.tile_pool(name="singles", bufs=1))
        ppro = st.enter_context(tc.tile_pool(name="ppro", bufs=2, space="PSUM"))
        xpool = st.enter_context(tc.tile_pool(name="xpool", bufs=2))
        epool = st.enter_context(tc.tile_pool(name="epool", bufs=2))
        opool = st.enter_context(tc.tile_pool(name="opool", bufs=8))
        pconv = st.enter_context(tc.tile_pool(name="pconv", bufs=6, space="PSUM"))
        dpool = st.enter_context(tc.tile_pool(name="dpool", bufs=1, space="DRAM"))

        # DRAM scratch for partition-rearranging bounces (dep-tracked pool tiles)
        cw_dram = dpool.tile([COUT * CIN * 27], F32, name="cw_dram")
        rw_dram = dpool.tile([E], F32, name="rw_dram")
        scl_dram = dpool.tile([COUT], F32, name="scl_dram")
        tf_dram = dpool.tile([COUT], F32, name="tf_dram")

        # ---------------- prologue: load small weights ----------------
        text_sb = singles.tile([128, 2], F32, name="text")
        nc.gpsimd.dma_start(out=text_sb, in_=text2.ap())
        rw1_sb = [singles.tile([128, HID], F32, name=f"rw1_{j}") for j in range(2)]
        mw1_sb = [singles.tile([128, HID], F32, name=f"mw1_{j}") for j in range(2)]
        for j in range(2):
            nc.gpsimd.dma_start(out=rw1_sb[j], in_=rw1.ap()[128 * j:128 * (j + 1), :])
            nc.gpsimd.dma_start(out=mw1_sb[j], in_=mw1.ap()[128 * j:128 * (j + 1), :])
        rb1_sb = singles.tile([HID, 1], F32, name="rb1")
        mb1_sb = singles.tile([HID, 1], F32, name="mb1")
        nc.gpsimd.dma_start(out=rb1_sb, in_=_pack_ap(rb1.ap(), 0, [(1, HID), (0, 1)]))
        nc.gpsimd.dma_start(out=mb1_sb, in_=_pack_ap(mb1.ap(), 0, [(1, HID), (0, 1)]))
        rw2_sb = singles.tile([HID, E], F32, name="rw2")
        mw2_sb = singles.tile([HID, COUT], F32, name="mw2")
        nc.gpsimd.dma_start(out=rw2_sb, in_=rw2.ap())
        nc.gpsimd.dma_start(out=mw2_sb, in_=mw2.ap())
        rb2_sb = singles.tile([1, E], F32, name="rb2")
        mb2_sb = singles.tile([1, COUT], F32, name="mb2")
        nc.gpsimd.dma_start(out=rb2_sb, in_=_pack_ap(rb2.ap(), 0, [(0, 1), (1, E)]))
        nc.gpsimd.dma_start(out=mb2_sb, in_=_pack_ap(mb2.ap(), 0, [(0, 1), (1, COUT)]))
        basis_sb = singles.tile([E, COUT * CIN * 27], F32, name="basis")
        nc.gpsimd.dma_start(out=basis_sb, in_=basis.ap())
        bbas_sb = singles.tile([E, COUT], F32, name="bbas")
        nc.gpsimd.dma_start(out=bbas_sb, in_=bbas.ap())

        # ---------------- router MLP ----------------
        h_ps = ppro.tile([HID, 1], F32, tag="pro", name="pro")
        nc.tensor.matmul(h_ps[:], rw1_sb[0][:], text_sb[:, 0:1], start=True, stop=False)
        nc.tensor.matmul(h_ps[:], rw1_sb[1][:], text_sb[:, 1:2], start=False, stop=True)
        # leaky_relu(z, 0.1) = max(z, 0.1 z)
        h_z = singles.tile([HID, 1], F32, name="h_z")
        nc.vector.tensor_add(h_z[:], h_ps[:], rb1_sb[:])
        h_s = singles.tile([HID, 1], F32, name="h_s")
        nc.vector.tensor_scalar_mul(h_s[:], h_z[:], 0.1)
        h_sb = singles.tile([HID, 1], F32, name="h")
        nc.vector.tensor_tensor(h_sb[:], h_z[:], h_s[:], mybir.AluOpType.max)

        g_ps = ppro.tile([HID, 1], F32, tag="pro", name="pro")
        nc.tensor.matmul(g_ps[:], mw1_sb[0][:], text_sb[:, 0:1], start=True, stop=False)
        nc.tensor.matmul(g_ps[:], mw1_sb[1][:], text_sb[:, 1:2], start=False, stop=True)
        g_z = singles.tile([HID, 1], F32, name="g_z")
        nc.vector.tensor_add(g_z[:], g_ps[:], mb1_sb[:])
        g_s = singles.tile([HID, 1], F32, name="g_s")
        nc.vector.tensor_scalar_mul(g_s[:], g_z[:], 0.1)
        g_sb = singles.tile([HID, 1], F32, name="g")
        nc.vector.tensor_tensor(g_sb[:], g_z[:], g_s[:], mybir.AluOpType.max)

        lg_ps = ppro.tile([1, E], F32, tag="pro", name="pro")
        nc.tensor.matmul(lg_ps[:], h_sb[:], rw2_sb[:], start=True, stop=True)
        lg_sb = singles.tile([1, E], F32, name="lg")
        nc.vector.tensor_add(lg_sb[:], lg_ps[:], rb2_sb[:])
        nc.scalar.dma_start(out=logits_d.ap(), in_=lg_sb[:])

        # softmax along free dim
        negmx = singles.tile([1, 1], F32, name="negmx")
        nc.vector.tensor_reduce(out=negmx[:], in_=lg_sb[:], op=mybir.AluOpType.max,
                                axis=mybir.AxisListType.X, negate=True)
        e_sb = singles.tile([1, E], F32, name="esb")
        nc.scalar.activation(e_sb[:], lg_sb[:], mybir.ActivationFunctionType.Exp,
                             bias=negmx[:], scale=1.0)
        ssum = singles.tile([1, 1], F32, name="ssum")
        nc.vector.reduce_sum(out=ssum[:], in_=e_sb[:], axis=mybir.AxisListType.X)
        rinv = singles.tile([1, 1], F32, name="rinv")
        nc.vector.reciprocal(rinv[:], ssum[:])
        rwf = singles.tile([1, E], F32, name="rwf")
        nc.vector.tensor_scalar_mul(rwf[:], e_sb[:], rinv[:])
        nc.scalar.dma_start(out=rwts_d.ap(), in_=rwf[:])

        # modulator -> scale = 1 + sigmoid(...)
        gm_ps = ppro.tile([1, COUT], F32, tag="pro", name="pro")
        nc.tensor.matmul(gm_ps[:], g_sb[:], mw2_sb[:], start=True, stop=True)
        gm_sb = singles.tile([1, COUT], F32, name="gm")
        nc.vector.tensor_add(gm_sb[:], gm_ps[:], mb2_sb[:])
        scl = singles.tile([1, COUT], F32, name="scl")
        nc.scalar.activation(scl[:], gm_sb[:], mybir.ActivationFunctionType.Sigmoid)
        nc.scalar.add(scl[:], scl[:], 1.0)

        # routing weights to partition layout [E, 1] via DRAM bounce
        nc.gpsimd.dma_start(out=rw_dram, in_=rwf[:])
        rwp = singles.tile([E, 1], F32, name="rwp")
        nc.gpsimd.dma_start(out=rwp[:], in_=_pack_ap(rw_dram, 0, [(1, E), (0, 1)]))

        # cb * scale  -> t_f [1, COUT]
        cb_ps = ppro.tile([1, COUT], F32, tag="pro", name="pro")
        nc.tensor.matmul(cb_ps[:], rwp[:], bbas_sb[:], start=True, stop=True)
        tf_sb = singles.tile([1, COUT], F32, name="tf")
        nc.vector.tensor_mul(tf_sb[:], cb_ps[:], scl[:])

        # per-partition scale/offset vectors [48, 1] (broadcast over q), via DRAM
        nc.gpsimd.dma_start(out=scl_dram, in_=scl[:])
        nc.gpsimd.dma_start(out=tf_dram, in_=tf_sb[:])
        scl48 = singles.tile([M, 1], F32, name="scl48")
        tf48 = singles.tile([M, 1], F32, name="tf48")
        for q in range(QZ):
            nc.gpsimd.dma_start(out=scl48[q * COUT:(q + 1) * COUT, 0:1],
                                in_=_pack_ap(scl_dram, 0, [(1, COUT), (0, 1)]))
            nc.gpsimd.dma_start(out=tf48[q * COUT:(q + 1) * COUT, 0:1],
                                in_=_pack_ap(tf_dram, 0, [(1, COUT), (0, 1)]))

        # mixed conv weights cw [1, 3456] -> DRAM
        cw_sb = singles.tile([1, COUT * CIN * 27], F32, name="cw")
        nmix = COUT * CIN * 27
        pos = 0
        while pos < nmix:
            n = min(512, nmix - pos)
            cw_ps = ppro.tile([1, 512], F32, tag="pro", name="pro")
            nc.tensor.matmul(cw_ps[:, 0:n], rwp[:], basis_sb[:, pos:pos + n],
                             start=True, stop=True)
            nc.scalar.copy(cw_sb[:, pos:pos + n], cw_ps[:, 0:n])
            pos += n
        nc.gpsimd.dma_start(out=cw_dram, in_=cw_sb[:])

        # banded lhsT tiles: w_v [128, 48] for v = (ty, tx)
        # w_v[(q+tz)*16 + i, q*8 + o] = cw[o*432 + i*27 + tz*9 + ty*3 + tx]
        wv = []
        for v in range(9):
            w = singles.tile([128, M], F32R, name=f"wv{v}")
            nc.gpsimd.memset(w[:].bitcast(F32), 0.0)
            wv.append(w)
        for ty in range(3):
            for tx in range(3):
                w = wv[ty * 3 + tx]
                for tz in range(3):
                    for q in range(QZ):
                        dst = w[(q + tz) * 16:(q + tz + 1) * 16,
                                q * COUT:(q + 1) * COUT]
                        src = _pack_ap(cw_dram, tz * 9 + ty * 3 + tx,
                                       [(27, CIN), (CIN * 27, COUT)])
                        nc.gpsimd.dma_start(out=dst, in_=src)
        # f32 copies of the x-edge variants (tx=0 and tx=2) for correction MMs
        wvf = {}
        for v in (0, 3, 6, 2, 5, 8):
            wf = singles.tile([128, M], F32, name=f"wvf{v}")
            nc.sync.dma_start(out=wf[:], in_=wv[v][:].bitcast(F32))
            wvf[v] = wf

        # ---------------- main conv ----------------
        for zb in range(ZB):
            xst = xpool.tile([128, XW], F32R, tag="xst", name="xst")
            nc.gpsimd.memset(xst[:, 0:G].bitcast(F32), 0.0)
            nc.gpsimd.memset(xst[:, G + SL:XW].bitcast(F32), 0.0)
            # stage 8 slices contiguously: partition p = zl*16 + i
            for zl in range(RZ):
                dst = xst[zl * 16:(zl + 1) * 16, G:G + SL]
                src = _pack_ap(xs.ap(), (QZ * zb + zl) * SL,
                               [(NSLICE * SL, CIN), (1, SL)])
                nc.sync.dma_start(out=dst, in_=src)

            # edge-column buffers (f32): E0 = x(:, :, y, 0), E95 = x(:, :, y, 95)
            e0 = epool.tile([128, EW], F32, tag="e0", name="e0")
            e95 = epool.tile([128, EW], F32, tag="e95", name="e95")
            nc.gpsimd.memset(e0[:, 0:EG], 0.0)
            nc.gpsimd.memset(e0[:, EG + D:EW], 0.0)
            nc.gpsimd.memset(e95[:, 0:EG], 0.0)
            nc.gpsimd.memset(e95[:, EG + D:EW], 0.0)
            nc.vector.tensor_copy(e0[:, EG:EG + D],
                                  _pack_ap(xst, G, [(XW, 128), (D, D)]).bitcast(F32))
            nc.vector.tensor_copy(e95[:, EG:EG + D],
                                  _pack_ap(xst, G + D - 1, [(XW, 128), (D, D)]).bitcast(F32))

            # wrap-error terms (f32 matmuls, N=96):
            # err0[m, y]  = sum_ty wvf[ty*3+0]^T @ E95[:, y+ty-2]
            # err95[m, y] = sum_ty wvf[ty*3+2]^T @ E0[:, y+ty]
            er0_ps = ppro.tile([M, D], F32, tag="pro", name="pro")
            for j, ty in enumerate(range(3)):
                rhs = _pack_ap(e95, EG + ty - 2, [(EW, 128), (1, D)])
                nc.tensor.matmul(er0_ps[:], wvf[ty * 3 + 0][:], rhs,
                                 start=(j == 0), stop=(j == 2))
            er95_ps = ppro.tile([M, D], F32, tag="pro", name="pro")
            for j, ty in enumerate(range(3)):
                rhs = _pack_ap(e0, EG + ty, [(EW, 128), (1, D)])
                nc.tensor.matmul(er95_ps[:], wvf[ty * 3 + 2][:], rhs,
                                 start=(j == 0), stop=(j == 2))
            er0_sb = epool.tile([M, D], F32, tag="er0", name="er0")
            er95_sb = epool.tile([M, D], F32, tag="er95", name="er95")
            nc.vector.tensor_copy(er0_sb[:], er0_ps[:])
            nc.vector.tensor_copy(er95_sb[:], er95_ps[:])

            for t0 in range(0, NT, TILESET):
                tset = range(t0, min(t0 + TILESET, NT))
                psums = {}
                for t in tset:
                    nr = min(RPT, D - t * RPT)
                    psums[t] = (pconv.tile([M, RPT * D], F32, tag="ps", name="ps"), nr)
                for v in range(9):
                    ty, tx = v // 3, v % 3
                    for t in tset:
                        ps, nr = psums[t]
                        n = nr * D
                        off = G + (t * RPT + ty - 1) * D + tx - 1
                        rhs = _pack_ap(xst, off, [(XW, 128), (1, n)])
                        nc.tensor.matmul(ps[:, 0:n], wv[v][:], rhs,
                                         start=(v == 0), stop=(v == 8))
                for t in tset:
                    ps, nr = psums[t]
                    n = nr * D
                    y0 = t * RPT
                    # subtract x-wrap errors on columns x=0 and x=95
                    nc.vector.tensor_tensor(
                        _pack_ap(ps, 0, [(RPT * D, M), (D, nr)]),
                        _pack_ap(ps, 0, [(RPT * D, M), (D, nr)]),
                        er0_sb[:, y0:y0 + nr], mybir.AluOpType.subtract)
                    nc.vector.tensor_tensor(
                        _pack_ap(ps, D - 1, [(RPT * D, M), (D, nr)]),
                        _pack_ap(ps, D - 1, [(RPT * D, M), (D, nr)]),
                        er95_sb[:, y0:y0 + nr], mybir.AluOpType.subtract)
                    # affine epilogue
                    ot = opool.tile([M, RPT * D], F32, tag="ot", name="ot")
                    nc.vector.tensor_scalar(
                        out=ot[:, 0:n], in0=ps[:, 0:n],
                        scalar1=scl48[:], scalar2=tf48[:],
                        op0=mybir.AluOpType.mult, op1=mybir.AluOpType.add)
                    src_o = _pack_ap(ot, 0, [(RPT * D, M), (1, n)])
                    dst_o = _pack_ap(out_d.ap(), (QZ * zb) * SL + y0 * D,
                                     [(SL, QZ), (SLAB * SL, COUT), (1, n)])
                    nc.scalar.dma_start(out=dst_o, in_=src_o)

    nc.compile()
    return nc


def _get_nc():
    if "nc" not in _CACHE:
        _CACHE["nc"] = _build_program()
    return _CACHE["nc"]


def kernel(x, text_features, weight_basis, bias_basis,
           r_w1, r_b1, r_w2, r_b2, m_w1, m_b1, m_w2, m_b2, **kw):
    x = np.ascontiguousarray(x, dtype=np.float32)
    nc = _get_nc()

    shared = {
        "r_w1": np.ascontiguousarray(r_w1, np.float32),
        "r_b1": np.ascontiguousarray(r_b1, np.float32),
        "r_w2": np.ascontiguousarray(r_w2, np.float32),
        "r_b2": np.ascontiguousarray(r_b2, np.float32),
        "m_w1": np.ascontiguousarray(m_w1, np.float32),
        "m_b1": np.ascontiguousarray(m_b1, np.float32),
        "m_w2": np.ascontiguousarray(m_w2, np.float32),
        "m_b2": np.ascontiguousarray(m_b2, np.float32),
        "basis": np.ascontiguousarray(
            np.asarray(weight_basis, np.float32).reshape(E, COUT * CIN * 27)),
        "bias_basis": np.ascontiguousarray(bias_basis, np.float32),
    }
    in_maps = []
    for c in range(NCORES):
        b, s = c // 4, c % 4
        slab = np.zeros((CIN, NSLICE, D, D), np.float32)
        lo, hi = SLAB * s - 1, SLAB * s + SLAB + 1
        vlo, vhi = max(0, lo), min(D, hi)
        slab[:, vlo - lo:vhi - lo] = x[b, :, vlo:vhi]
        m = dict(shared)
        m["x_slab"] = slab
        m["text2"] = np.ascontiguousarray(
            np.asarray(text_features[b], np.float32).reshape(2, 128).T)
        in_maps.append(m)

    res = run_bass_kernel_spmd(nc, in_maps, core_ids=list(range(NCORES)),
                               **_CACHE.get("run_kwargs", {}))
    _CACHE["last_results"] = res

    out = np.empty((B, COUT, D, D, D), np.float32)
    logits = np.empty((B, E), np.float32)
    rwts = np.empty((B, E), np.float32)
    for c in range(NCORES):
        b, s = c // 4, c % 4
        out[b, :, SLAB * s:SLAB * (s + 1)] = res.results[c]["out"]
        if s == 0:
            logits[b] = res.results[c]["logits"][0]
            rwts[b] = res.results[c]["rweights"][0]
    return out, logits, rwts
